# revision 7
# baseline (speedup 1.0000x reference)
"""Trainium2 Bass kernel for nn_DeepDownstreamFork (dense MLP chain + skips + layernorm).

Reference computation (per batch b of 8, each handled by one NeuronCore):
    big = relu(x @ (W_large * S_large).T)          # [T, L]   T=4096, H=1024, L=4096
    big = relu(big @ W_c1.T)                        # [T, L]
    big = big @ W_c2.T                              # [T, H]
    s   = x @ (W_s1 * S_s1 + W_s2 * S_s2).T         # [T, H]  (s1+s2 folded)
    out = layernorm(big + s) * gamma + beta         # [T, H]

Strategy: pure data-parallel over the batch dim (8 batches -> 8 cores, zero
communication).  All matmuls run in split-fp8: each operand X is decomposed
as X ~ Xh + Xl with Xh = e4m3(X*s), Xl = e4m3(X*s - Xh) at the SAME power-2
scale s, and the product uses three fp8 DoubleRow passes
(Ah*Wh + Ah*Wl + Al*Wh, dropping the Al*Wl term).  DoubleRow contracts 256
rows per instruction at 2x bf16 throughput, so the three passes cost 0.75x
of a bf16 schedule while keeping ~bf16 accuracy (2.5e-3 rel measured).

Schedule is tile-major with fused A->B: per 512-token tile, phase A (large
proj) uses resident WlT fp8 pairs, y1 pairs stay in SBUF, then phase B
streams Wc1T fp8 pair chunks from a DRAM cache (prepared on the fly during
tile 0) and spills y2 pairs to DRAM.  Phase C (chain_2 + skips + layernorm)
runs after all AB tiles with resident Wc2T/WsT pairs (these reuse the WlT
SBUF slots via tag rotation), reloading y2/xT pairs per tile.  Weight
transposes go through the DMA xbar on bf16 stagings (SBUF->SBUF for
W_large/W_c1, DRAM for W_c2/W_s), overlapped with compute.
"""

import os
import sys

import numpy as np

if os.path.isdir("/opt/trn_rl_repo") and "/opt/trn_rl_repo" not in sys.path:
    sys.path.insert(0, "/opt/trn_rl_repo")

P = 128
H = 1024          # hidden
L = 4096          # large dim
T = 4096          # tokens per core (batch dim sharded across cores)
NCORES = 8
TT = 512          # token tile
HBLK = H // P     # 8
LBLK = L // P     # 32

# power-2 quantization scales
SX = 32.0         # x
SWL = 32.0        # W_large
SWC1 = 1024.0     # W_c1
SWC2 = 1024.0     # W_c2
SWS = 16.0        # W_s1+W_s2
EA = 2.0 ** -11   # A-psum evac: (x*32)*(WL*32) = y1*1024 -> store y1/2
EB = 2.0 ** -10   # B-psum evac: (y1/2)*(Wc1*1024) = y2*512 -> store y2/2
EC = 2.0 ** -9    # C-psum evac: (y2/2)*(Wc2*1024) + (x*32)*(Ws*16) = (y3+s)*512

_CACHED = {}


def _build_nc(ntt=T // TT, identity_ln=True, debug=False):
    import concourse.mybir as mybir
    import concourse.tile as tile
    from concourse import bacc

    F32 = mybir.dt.float32
    BF16 = mybir.dt.bfloat16
    FP16 = mybir.dt.float16
    FP8 = mybir.dt.float8e4
    Relu = mybir.ActivationFunctionType.Relu
    Copy = mybir.ActivationFunctionType.Copy
    Sqrt = mybir.ActivationFunctionType.Sqrt
    MUL = mybir.AluOpType.mult
    SUB = mybir.AluOpType.subtract
    ADD = mybir.AluOpType.add
    MAX = mybir.AluOpType.max
    DR = mybir.MatmulPerfMode.DoubleRow

    Tn = ntt * TT

    nc = bacc.Bacc(
        None, target_bir_lowering=False, debug=debug,
        dynamic_dma_scratch_size=32768,
    )

    x = nc.dram_tensor("x", [Tn, H], F32, kind="ExternalInput")
    W_large = nc.dram_tensor("W_large", [L, H], F32, kind="ExternalInput")
    W_s1 = nc.dram_tensor("W_s1", [H, H], F32, kind="ExternalInput")
    W_s2 = nc.dram_tensor("W_s2", [H, H], F32, kind="ExternalInput")
    W_c1 = nc.dram_tensor("W_c1", [L, L], F32, kind="ExternalInput")
    W_c2 = nc.dram_tensor("W_c2", [H, L], F32, kind="ExternalInput")
    gamma = nc.dram_tensor("gamma", [H], F32, kind="ExternalInput")
    beta = nc.dram_tensor("beta", [H], F32, kind="ExternalInput")
    s_large = nc.dram_tensor("s_large", [L // P, H // P], F32, kind="ExternalInput")
    s_s1 = nc.dram_tensor("s_s1", [H // P, H // P], F32, kind="ExternalInput")
    s_s2 = nc.dram_tensor("s_s2", [H // P, H // P], F32, kind="ExternalInput")
    out = nc.dram_tensor("out", [Tn, H], F32, kind="ExternalOutput")

    with tile.TileContext(nc) as tc:
        with (
            tc.tile_pool(name="dram", bufs=1, space="DRAM") as dram,
            tc.tile_pool(name="consts", bufs=1) as consts,
            tc.tile_pool(name="wbig", bufs=1) as wbig,
            tc.tile_pool(name="wsp", bufs=1) as wsp,
            tc.tile_pool(name="wprep", bufs=2) as wprep,
            tc.tile_pool(name="xp", bufs=1) as xp,
            tc.tile_pool(name="y1p", bufs=1) as y1p,
            tc.tile_pool(name="wc1s", bufs=2) as wc1s,
            tc.tile_pool(name="stg", bufs=3) as stg,
            tc.tile_pool(name="lnp", bufs=2) as lnp,
            tc.tile_pool(name="psum", bufs=6, space="PSUM") as psum,
        ):
            # ---- DRAM scratch ----
            x_bf = dram.tile([Tn, H], BF16)
            Wc2_bf = dram.tile([H, L], BF16)
            Ws_bf = dram.tile([H, H], BF16)
            WC1H = dram.tile([LBLK, P, LBLK, P], FP8)   # per l2-chunk: [l1p, l1blk, l2]
            WC1L = dram.tile([LBLK, P, LBLK, P], FP8)
            Y2H = dram.tile([ntt, P, LBLK, TT], FP8)
            Y2L = dram.tile([ntt, P, LBLK, TT], FP8)
            XTH = dram.tile([ntt, P, HBLK, TT], FP8)
            XTL = dram.tile([ntt, P, HBLK, TT], FP8)

            # ---- early SWDGE casts (gpsimd queue) ----
            for s in range(ntt):
                nc.gpsimd.dma_start(x_bf[s * TT:(s + 1) * TT, :], x[s * TT:(s + 1) * TT, :])

            # ---- constants ----
            sc_l = consts.tile([P, LBLK * HBLK], F32)
            nc.sync.dma_start(
                sc_l[:], s_large[:].rearrange("a b -> (a b)")[None, :].to_broadcast([P, LBLK * HBLK])
            )
            nc.vector.tensor_scalar_mul(sc_l[:], sc_l[:], SWL)
            sc_s1 = consts.tile([P, HBLK * HBLK], F32)
            nc.sync.dma_start(
                sc_s1[:], s_s1[:].rearrange("a b -> (a b)")[None, :].to_broadcast([P, HBLK * HBLK])
            )
            nc.vector.tensor_scalar_mul(sc_s1[:], sc_s1[:], SWS)
            sc_s2 = consts.tile([P, HBLK * HBLK], F32)
            nc.sync.dma_start(
                sc_s2[:], s_s2[:].rearrange("a b -> (a b)")[None, :].to_broadcast([P, HBLK * HBLK])
            )
            nc.vector.tensor_scalar_mul(sc_s2[:], sc_s2[:], SWS)
            if not identity_ln:
                gamma_b = consts.tile([P, H], FP16)
                nc.sync.dma_start(gamma_b[:], gamma[:][None, :].to_broadcast([P, H]))
                beta_b = consts.tile([P, H], FP16)
                nc.sync.dma_start(beta_b[:], beta[:][None, :].to_broadcast([P, H]))
            eps_t = consts.tile([P, 1], F32)
            nc.vector.memset(eps_t[:], 1e-5)

            # ---- resident fp8 weight pair tiles ----
            # WlT hi/lo split at h-pair boundary: [h-part, 4 h-blocks, l]
            WlTh = [wbig.tile([P, 4, L], FP8, tag=f"wbig{i}", name=f"WlTh{i}") for i in range(2)]
            WlTl = [wbig.tile([P, 4, L], FP8, tag=f"wbig{i+2}", name=f"WlTl{i}") for i in range(2)]

            def wl_stat(level, k, msl):
                # stationary pair AP for h-pair k (k in 0..3), l columns msl
                tl = (WlTh if level == 0 else WlTl)[k // 2]
                return tl[:, 2 * (k % 2):2 * (k % 2) + 2, msl]

            def prep_wl_chunk(m):
                # one 128-row chunk of W_large -> WlT fp8 hi/lo at columns m*P
                wtmp = wprep.tile([P, H], F32, tag="wtmp", name="wtmp")
                nc.sync.dma_start(wtmp[:], W_large[m * P:(m + 1) * P, :])
                wbf = wprep.tile([P, H], BF16, tag="wbf", name="wbf")
                nc.vector.tensor_tensor(
                    wbf[:].rearrange("p (b c) -> p b c", c=P),
                    wtmp[:].rearrange("p (b c) -> p b c", c=P),
                    sc_l[:, m * HBLK:(m + 1) * HBLK, None].to_broadcast([P, HBLK, P]),
                    MUL,
                )
                wtb = wprep.tile([P, HBLK, P], BF16, tag="wtb", name="wtb", bufs=1)
                nc.sync.dma_start_transpose(wtb[:], wbf[:])
                for i in range(2):
                    hsl = wtb[:, 4 * i:4 * (i + 1), :]
                    hi = WlTh[i][:, :, m * P:(m + 1) * P]
                    nc.scalar.activation(hi, hsl, Copy)
                    nc.vector.tensor_tensor(
                        WlTl[i][:, :, m * P:(m + 1) * P], hsl, hi, SUB)

            def prep_wc1_chunk(c):
                # one 128-row (l2) chunk of W_c1 -> DRAM fp8 pair cache, in quarters
                Q = 1024
                for q in range(4):
                    wtmp = wprep.tile([P, Q], F32, tag="wtmp", name="c1tmp")
                    nc.sync.dma_start(wtmp[:], W_c1[c * P:(c + 1) * P, q * Q:(q + 1) * Q])
                    wbf = wprep.tile([P, Q], BF16, tag="wbf", name="c1bf")
                    nc.vector.tensor_scalar_mul(wbf[:], wtmp[:], SWC1)
                    wtb = wprep.tile([P, Q // P, P], BF16, tag="wtb", name="c1tb", bufs=1)
                    nc.sync.dma_start_transpose(wtb[:], wbf[:])
                    hi = wprep.tile([P, Q // P, P], FP8, tag="whi", name="c1hi", bufs=1)
                    nc.scalar.activation(hi[:], wtb[:], Copy)
                    lo = wprep.tile([P, Q // P, P], FP8, tag="wlo", name="c1lo", bufs=1)
                    nc.vector.tensor_tensor(lo[:], wtb[:], hi[:], SUB)
                    nc.sync.dma_start(WC1H[c][:, q * (Q // P):(q + 1) * (Q // P), :], hi[:])
                    nc.sync.dma_start(WC1L[c][:, q * (Q // P):(q + 1) * (Q // P), :], lo[:])

            def prep_wc2_bf():
                # W_c2 f32 -> bf16*SWC2 DRAM scratch (for DRAM->SBUF transposes)
                for ch in range(H // P):
                    for hf in range(4):
                        wtmp = wprep.tile([P, L // 4], F32, tag="wtmp", name="c2tmp")
                        nc.sync.dma_start(
                            wtmp[:], W_c2[ch * P:(ch + 1) * P, hf * (L // 4):(hf + 1) * (L // 4)])
                        wbf = wprep.tile([P, L // 4], BF16, tag="wbf", name="c2bf")
                        nc.vector.tensor_scalar_mul(wbf[:], wtmp[:], SWC2)
                        nc.sync.dma_start(
                            Wc2_bf[ch * P:(ch + 1) * P, hf * (L // 4):(hf + 1) * (L // 4)], wbf[:])

            def prep_ws_bf():
                # Ws = W_s1*s_s1*SWS + W_s2*s_s2*SWS -> bf16 DRAM scratch
                for ch in range(H // P):
                    w1 = wprep.tile([P, H], F32, tag="wtmp", name="ws1")
                    nc.sync.dma_start(w1[:], W_s1[ch * P:(ch + 1) * P, :])
                    w2 = wprep.tile([P, H], F32, tag="wtmp2", name="ws2", bufs=1)
                    nc.sync.dma_start(w2[:], W_s2[ch * P:(ch + 1) * P, :])
                    t1 = wprep.tile([P, H], BF16, tag="wbf", name="wst1")
                    nc.vector.tensor_tensor(
                        t1[:].rearrange("p (b c) -> p b c", c=P),
                        w1[:].rearrange("p (b c) -> p b c", c=P),
                        sc_s1[:, ch * HBLK:(ch + 1) * HBLK, None].to_broadcast([P, HBLK, P]),
                        MUL,
                    )
                    t2 = wprep.tile([P, H], BF16, tag="wbf2", name="wst2", bufs=1)
                    nc.vector.tensor_tensor(
                        t2[:].rearrange("p (b c) -> p b c", c=P),
                        w2[:].rearrange("p (b c) -> p b c", c=P),
                        sc_s2[:, ch * HBLK:(ch + 1) * HBLK, None].to_broadcast([P, HBLK, P]),
                        MUL,
                    )
                    ts = wprep.tile([P, H], BF16, tag="wbf3", name="wsts", bufs=1)
                    nc.vector.tensor_tensor(ts[:], t1[:], t2[:], ADD)
                    nc.sync.dma_start(Ws_bf[ch * P:(ch + 1) * P, :], ts[:])

            # C-phase resident tiles (reuse wbig slots after A is done with WlT)
            W2h = [None, None]
            W2l = [None, None]
            WsTh = [None, None]
            WsTl = [None, None]

            def prep_wc2T(hh):
                # transpose+quantize Wc2 half hh: moving [l2p, l2blk, 512 h]
                W2h[hh] = wbig.tile([P, LBLK, TT], FP8, tag=f"wbig{2*hh}", name=f"W2h{hh}")
                W2l[hh] = wbig.tile([P, LBLK, TT], FP8, tag=f"wbig{2*hh+1}", name=f"W2l{hh}")
                for lf in range(4):  # split transpose by l2 quarters to bound staging
                    wtb = wprep.tile([P, HBLK, TT], BF16, tag="stg16", name="c2T", bufs=1)
                    nc.sync.dma_start_transpose(
                        wtb[:], Wc2_bf[hh * TT:(hh + 1) * TT, lf * (L // 4):(lf + 1) * (L // 4)])
                    hsl = W2h[hh][:, lf * HBLK:(lf + 1) * HBLK, :]
                    nc.scalar.activation(hsl, wtb[:], Copy)
                    nc.vector.tensor_tensor(
                        W2l[hh][:, lf * HBLK:(lf + 1) * HBLK, :], wtb[:], hsl, SUB)

            def prep_wsT(hh):
                WsTh[hh] = wsp.tile([P, HBLK, TT], FP8, tag=f"wsp{2*hh}", name=f"WsTh{hh}")
                WsTl[hh] = wsp.tile([P, HBLK, TT], FP8, tag=f"wsp{2*hh+1}", name=f"WsTl{hh}")
                wtb = wprep.tile([P, HBLK, TT], BF16, tag="stg16", name="wsT", bufs=1)
                nc.sync.dma_start_transpose(wtb[:], Ws_bf[hh * TT:(hh + 1) * TT, :])
                nc.scalar.activation(WsTh[hh][:], wtb[:], Copy)
                nc.vector.tensor_tensor(WsTl[hh][:], wtb[:], WsTh[hh][:], SUB)

            # ================= AB tiles =================
            for t in range(ntt):
                first = t == 0
                par = t % 2

                # x transpose + quantize for this tile
                xtb = wprep.tile([P, HBLK, TT], BF16, tag="stg16", name="xtb", bufs=1)
                nc.sync.dma_start_transpose(xtb[:], x_bf[t * TT:(t + 1) * TT, :])
                xth = xp.tile([P, HBLK, TT], FP8, tag=f"xh{par}", name="xth")
                nc.scalar.activation(xth[:], xtb[:], Copy, scale=SX)
                xtl = xp.tile([P, HBLK, TT], FP8, tag=f"xl{par}", name="xtl")
                nc.vector.scalar_tensor_tensor(xtl[:], xtb[:], SX, xth[:], MUL, SUB)
                nc.sync.dma_start(XTH[t], xth[:])
                nc.sync.dma_start(XTL[t], xtl[:])

                # ---- phase A: y1T = relu(WlT.T x) ----
                y1h = y1p.tile([P, LBLK, TT], FP8, tag="y1h", name="y1h")
                y1l = y1p.tile([P, LBLK, TT], FP8, tag="y1l", name="y1l")
                with nc.named_scope(f"A{t}"):
                    for m in range(LBLK):
                        if first:
                            prep_wl_chunk(m)
                        msl = slice(m * P, (m + 1) * P)
                        ps = psum.tile([P, TT], F32, tag="ps", name="psA")
                        for k in range(4):
                            nc.tensor.matmul(
                                ps[:], wl_stat(0, k, msl), xth[:, 2 * k:2 * k + 2, :],
                                start=(k == 0), stop=False, perf_mode=DR)
                            nc.tensor.matmul(
                                ps[:], wl_stat(0, k, msl), xtl[:, 2 * k:2 * k + 2, :],
                                start=False, stop=False, perf_mode=DR)
                        for k in range(4):
                            nc.tensor.matmul(
                                ps[:], wl_stat(1, k, msl), xth[:, 2 * k:2 * k + 2, :],
                                start=False, stop=(k == 3), perf_mode=DR)
                        hi = y1h[:, m, :]
                        nc.scalar.activation(hi, ps[:], Relu, scale=EA)
                        tst = stg.tile([P, TT], FP16, tag="tst", name="tstA")
                        nc.vector.tensor_scalar(tst[:], ps[:], EA, 0.0, op0=MUL, op1=MAX)
                        nc.vector.tensor_tensor(y1l[:, m, :], tst[:], hi, SUB)

                # ---- phase B: y2T = relu(Wc1T.T y1) ----
                with nc.named_scope(f"B{t}"):
                    for c in range(LBLK):
                        if first:
                            prep_wc1_chunk(c)
                        if t == min(1, ntt - 1) and c == 0:
                            prep_wc2_bf()
                        if t == max(ntt - 2, 0) and c == 16:
                            prep_ws_bf()
                        if t == ntt - 1:
                            if c == 0:
                                prep_wc2T(0)
                            if c == 8:
                                prep_wc2T(1)
                            if c == 16:
                                prep_wsT(0)
                            if c == 20:
                                prep_wsT(1)
                        w1h = wc1s.tile([P, LBLK, P], FP8, tag="w1h", name="w1h")
                        nc.sync.dma_start(w1h[:], WC1H[c])
                        w1l = wc1s.tile([P, LBLK, P], FP8, tag="w1l", name="w1l")
                        nc.sync.dma_start(w1l[:], WC1L[c])
                        ps = psum.tile([P, TT], F32, tag="ps", name="psB")
                        for j in range(16):
                            nc.tensor.matmul(
                                ps[:], w1h[:, 2 * j:2 * j + 2, :], y1h[:, 2 * j:2 * j + 2, :],
                                start=(j == 0), stop=False, perf_mode=DR)
                            nc.tensor.matmul(
                                ps[:], w1h[:, 2 * j:2 * j + 2, :], y1l[:, 2 * j:2 * j + 2, :],
                                start=False, stop=False, perf_mode=DR)
                        for j in range(16):
                            nc.tensor.matmul(
                                ps[:], w1l[:, 2 * j:2 * j + 2, :], y1h[:, 2 * j:2 * j + 2, :],
                                start=False, stop=(j == 15), perf_mode=DR)
                        y2hs = stg.tile([P, TT], FP8, tag="y2h", name="y2hs")
                        nc.scalar.activation(y2hs[:], ps[:], Relu, scale=EB)
                        tst = stg.tile([P, TT], FP16, tag="tst", name="tstB")
                        nc.vector.tensor_scalar(tst[:], ps[:], EB, 0.0, op0=MUL, op1=MAX)
                        y2ls = stg.tile([P, TT], FP8, tag="y2l", name="y2ls")
                        nc.vector.tensor_tensor(y2ls[:], tst[:], y2hs[:], SUB)
                        nc.sync.dma_start(Y2H[t][:, c, :], y2hs[:])
                        nc.sync.dma_start(Y2L[t][:, c, :], y2ls[:])

            # ================= C tiles: y3 + skips + layernorm =================
            for t in range(ntt):
                par = t % 2
                y2h = y1p.tile([P, LBLK, TT], FP8, tag="y1h", name="cy2h")
                nc.sync.dma_start(y2h[:], Y2H[t])
                y2l = y1p.tile([P, LBLK, TT], FP8, tag="y1l", name="cy2l")
                nc.sync.dma_start(y2l[:], Y2L[t])
                xh = xp.tile([P, HBLK, TT], FP8, tag=f"xh{par}", name="cxh")
                nc.sync.dma_start(xh[:], XTH[t])
                xl = xp.tile([P, HBLK, TT], FP8, tag=f"xl{par}", name="cxl")
                nc.sync.dma_start(xl[:], XTL[t])
                with nc.named_scope(f"C{t}"):
                    for tn in range(TT // P):
                        tsl = slice(tn * P, (tn + 1) * P)
                        preln = lnp.tile([P, H], FP16, tag="preln", name="preln", bufs=2)
                        pss = [psum.tile([P, TT], F32, tag="ps", name="psC")
                               for _ in range(2)]
                        for j in range(16):
                            for hh, ps in enumerate(pss):
                                nc.tensor.matmul(
                                    ps[:], y2h[:, 2 * j:2 * j + 2, tsl],
                                    W2h[hh][:, 2 * j:2 * j + 2, :],
                                    start=(j == 0), stop=False, perf_mode=DR)
                                nc.tensor.matmul(
                                    ps[:], y2h[:, 2 * j:2 * j + 2, tsl],
                                    W2l[hh][:, 2 * j:2 * j + 2, :],
                                    start=False, stop=False, perf_mode=DR)
                        for j in range(16):
                            for hh, ps in enumerate(pss):
                                nc.tensor.matmul(
                                    ps[:], y2l[:, 2 * j:2 * j + 2, tsl],
                                    W2h[hh][:, 2 * j:2 * j + 2, :],
                                    start=False, stop=False, perf_mode=DR)
                        for k in range(4):
                            for hh, ps in enumerate(pss):
                                nc.tensor.matmul(
                                    ps[:], xh[:, 2 * k:2 * k + 2, tsl],
                                    WsTh[hh][:, 2 * k:2 * k + 2, :],
                                    start=False, stop=False, perf_mode=DR)
                                nc.tensor.matmul(
                                    ps[:], xh[:, 2 * k:2 * k + 2, tsl],
                                    WsTl[hh][:, 2 * k:2 * k + 2, :],
                                    start=False, stop=False, perf_mode=DR)
                        for k in range(4):
                            for hh, ps in enumerate(pss):
                                nc.tensor.matmul(
                                    ps[:], xl[:, 2 * k:2 * k + 2, tsl],
                                    WsTh[hh][:, 2 * k:2 * k + 2, :],
                                    start=False, stop=(k == 3), perf_mode=DR)
                        for hh, ps in enumerate(pss):
                            nc.scalar.activation(
                                preln[:, hh * TT:(hh + 1) * TT], ps[:], Copy, scale=EC)
                        # layernorm over h
                        st = lnp.tile([P, 2, 6], F32, tag="st", name="st")
                        for g in range(2):
                            nc.vector.bn_stats(st[:, g, :], preln[:, g * 512:(g + 1) * 512])
                        mv = lnp.tile([P, 2], F32, tag="mv", name="mv")
                        nc.vector.bn_aggr(mv[:], st[:])
                        std = lnp.tile([P, 1], F32, tag="std", name="std")
                        nc.scalar.activation(std[:], mv[:, 1:2], Sqrt, bias=eps_t[:])
                        rstd = lnp.tile([P, 1], F32, tag="rstd", name="rstd")
                        nc.vector.reciprocal(rstd[:], std[:])
                        og = lnp.tile([P, H], F32, tag="og", name="og", bufs=1)
                        if identity_ln:
                            nc.vector.tensor_scalar(
                                og[:], preln[:], scalar1=mv[:, 0:1], scalar2=rstd[:],
                                op0=SUB, op1=MUL)
                        else:
                            nc.vector.tensor_scalar(
                                preln[:], preln[:], scalar1=mv[:, 0:1], scalar2=rstd[:],
                                op0=SUB, op1=MUL)
                            nc.vector.tensor_tensor(preln[:], preln[:], gamma_b[:], MUL)
                            nc.vector.tensor_tensor(og[:], preln[:], beta_b[:], ADD)
                        nc.sync.dma_start(out[t * TT + tn * P:t * TT + (tn + 1) * P, :], og[:])

    nc.finalize()
    return nc


def _get_nc(identity_ln=True):
    key = (T // TT, identity_ln)
    if key not in _CACHED:
        _CACHED[key] = _build_nc(T // TT, identity_ln)
    return _CACHED[key]


def kernel(**inputs) -> np.ndarray:
    from concourse.bass_utils import run_bass_kernel_spmd

    x = np.asarray(inputs["x"], dtype=np.float32)          # [8, 4096, 1024]
    gamma_np = np.asarray(inputs["gamma"], dtype=np.float32)
    beta_np = np.asarray(inputs["beta"], dtype=np.float32)
    identity_ln = bool(np.all(gamma_np == 1.0) and np.all(beta_np == 0.0))
    nc = _get_nc(identity_ln)
    shared_names = (
        "W_large", "W_s1", "W_s2", "W_c1", "W_c2",
        "gamma", "beta", "s_large", "s_s1", "s_s2",
    )
    shared = {k: np.ascontiguousarray(np.asarray(inputs[k], dtype=np.float32))
              for k in shared_names}
    in_maps = [
        {"x": np.ascontiguousarray(x[c]), **shared} for c in range(NCORES)
    ]
    res = run_bass_kernel_spmd(nc, in_maps, core_ids=list(range(NCORES)))
    return np.stack([res.results[c]["out"] for c in range(NCORES)], axis=0)


# revision 9
# speedup vs baseline: 1.4636x; 1.4636x over previous
"""Trainium2 Bass kernel for nn_DeepDownstreamFork (dense MLP chain + skips + layernorm).

Reference computation (per batch b of 8, each handled by one NeuronCore):
    big = relu(x @ (W_large * S_large).T)          # [T, L]   T=4096, H=1024, L=4096
    big = relu(big @ W_c1.T)                        # [T, L]
    big = big @ W_c2.T                              # [T, H]
    s   = x @ (W_s1 * S_s1 + W_s2 * S_s2).T         # [T, H]  (s1+s2 folded)
    out = layernorm(big + s) * gamma + beta         # [T, H]

Strategy: pure data-parallel over the batch dim (8 batches -> 8 cores, zero
communication).  Matmuls run in bf16 except a tunable upper fraction of
phase B's contraction (the dominant 64% of FLOPs), which runs in plain fp8
e4m3 using DoubleRow mode with byte-interleaved operand pairs -- measured
2x bf16 throughput on the PE.  NLOW of 32 contraction blocks stay bf16;
the remaining NUP blocks are fp8 (error scales ~1.95e-2 * sqrt(NUP/32),
well under the 2e-2 gate; bf16-only is 2.7e-3).

Schedule is tile-major with fused A->B: per 512-token tile, phase A uses a
resident bf16 WlT (transposed on the fly via SBUF->SBUF xbar during tile
0), y1 stays in SBUF (bf16 lower blocks + interleaved fp8 upper), then
phase B streams Wc1T chunks from a DRAM cache (bf16 lower + fp8 upper,
prepared during tile 0) and spills y2 bf16 to DRAM.  Phase C (chain_2 +
skips + layernorm, all bf16) runs after all AB tiles with resident
Wc2T/WsT (Wc2T reuses WlT's SBUF slot), reloading y2 and re-transposing
xT per tile.
"""

import os
import sys

import numpy as np

if os.path.isdir("/opt/trn_rl_repo") and "/opt/trn_rl_repo" not in sys.path:
    sys.path.insert(0, "/opt/trn_rl_repo")

P = 128
H = 1024          # hidden
L = 4096          # large dim
T = 4096          # tokens per core (batch dim sharded across cores)
NCORES = 8
TT = 512          # token tile
HBLK = H // P     # 8
LBLK = L // P     # 32

NLOW = 12         # B-contraction blocks kept bf16 (of 32); rest fp8 DoubleRow
NUP = LBLK - NLOW # fp8 blocks (must be even)
SWC1 = 1024.0     # Wc1 scale for both bf16 (exact) and fp8 range
SWC2 = 1024.0
SWS = 512.0
EA = 0.5          # A evac: store y1/2 (fp8 range: y1 max ~219 -> 110)
EB = 2.0 ** -10   # B evac: psum = (y1/2)*(Wc1*1024) = 512*y2 -> store y2/2
EC = 2.0 ** -9    # C evac: psum = (y2/2)*(Wc2*1024) + x*(Ws*512) = 512*(y3+s)

_CACHED = {}


def _build_nc(ntt=T // TT, identity_ln=True, debug=False):
    import concourse.mybir as mybir
    import concourse.tile as tile
    from concourse import bacc

    F32 = mybir.dt.float32
    BF16 = mybir.dt.bfloat16
    FP16 = mybir.dt.float16
    FP8 = mybir.dt.float8e4
    Relu = mybir.ActivationFunctionType.Relu
    Copy = mybir.ActivationFunctionType.Copy
    Sqrt = mybir.ActivationFunctionType.Sqrt
    MUL = mybir.AluOpType.mult
    SUB = mybir.AluOpType.subtract
    ADD = mybir.AluOpType.add
    DR = mybir.MatmulPerfMode.DoubleRow

    Tn = ntt * TT

    nc = bacc.Bacc(
        None, target_bir_lowering=False, debug=debug,
        dynamic_dma_scratch_size=32768,
    )

    x = nc.dram_tensor("x", [Tn, H], F32, kind="ExternalInput")
    W_large = nc.dram_tensor("W_large", [L, H], F32, kind="ExternalInput")
    W_s1 = nc.dram_tensor("W_s1", [H, H], F32, kind="ExternalInput")
    W_s2 = nc.dram_tensor("W_s2", [H, H], F32, kind="ExternalInput")
    W_c1 = nc.dram_tensor("W_c1", [L, L], F32, kind="ExternalInput")
    W_c2 = nc.dram_tensor("W_c2", [H, L], F32, kind="ExternalInput")
    gamma = nc.dram_tensor("gamma", [H], F32, kind="ExternalInput")
    beta = nc.dram_tensor("beta", [H], F32, kind="ExternalInput")
    s_large = nc.dram_tensor("s_large", [L // P, H // P], F32, kind="ExternalInput")
    s_s1 = nc.dram_tensor("s_s1", [H // P, H // P], F32, kind="ExternalInput")
    s_s2 = nc.dram_tensor("s_s2", [H // P, H // P], F32, kind="ExternalInput")
    out = nc.dram_tensor("out", [Tn, H], F32, kind="ExternalOutput")

    with tile.TileContext(nc) as tc:
        with (
            tc.tile_pool(name="dram", bufs=1, space="DRAM") as dram,
            tc.tile_pool(name="consts", bufs=1) as consts,
            tc.tile_pool(name="wbig", bufs=1) as wbig,
            tc.tile_pool(name="wsp", bufs=1) as wsp,
            tc.tile_pool(name="wprep", bufs=2) as wprep,
            tc.tile_pool(name="xp", bufs=1) as xp,
            tc.tile_pool(name="yp", bufs=1) as yp,
            tc.tile_pool(name="wc1s", bufs=2) as wc1s,
            tc.tile_pool(name="stg", bufs=3) as stg,
            tc.tile_pool(name="lnp", bufs=2) as lnp,
            tc.tile_pool(name="psum", bufs=6, space="PSUM") as psum,
        ):
            # ---- DRAM scratch ----
            x_bf = dram.tile([Tn, H], BF16)
            Wc2_bf = dram.tile([H, L], BF16)
            Ws_bf = dram.tile([H, H], BF16)
            WC1B = dram.tile([LBLK, P, NLOW, P], BF16)  # streamed bf16 stationary
            WC1U = dram.tile([LBLK, P, NUP, P], FP8)    # streamed fp8 stationary
            Y2B = dram.tile([ntt, P, LBLK, TT], BF16)

            # ---- early SWDGE casts (gpsimd queue) ----
            for s in range(ntt):
                nc.gpsimd.dma_start(x_bf[s * TT:(s + 1) * TT, :], x[s * TT:(s + 1) * TT, :])

            # ---- constants ----
            sc_l = consts.tile([P, LBLK * HBLK], F32)
            nc.sync.dma_start(
                sc_l[:], s_large[:].rearrange("a b -> (a b)")[None, :].to_broadcast([P, LBLK * HBLK])
            )
            sc_s1 = consts.tile([P, HBLK * HBLK], F32)
            nc.sync.dma_start(
                sc_s1[:], s_s1[:].rearrange("a b -> (a b)")[None, :].to_broadcast([P, HBLK * HBLK])
            )
            nc.vector.tensor_scalar_mul(sc_s1[:], sc_s1[:], SWS)
            sc_s2 = consts.tile([P, HBLK * HBLK], F32)
            nc.sync.dma_start(
                sc_s2[:], s_s2[:].rearrange("a b -> (a b)")[None, :].to_broadcast([P, HBLK * HBLK])
            )
            nc.vector.tensor_scalar_mul(sc_s2[:], sc_s2[:], SWS)
            if not identity_ln:
                gamma_b = consts.tile([P, H], FP16)
                nc.sync.dma_start(gamma_b[:], gamma[:][None, :].to_broadcast([P, H]))
                beta_b = consts.tile([P, H], FP16)
                nc.sync.dma_start(beta_b[:], beta[:][None, :].to_broadcast([P, H]))
            eps_t = consts.tile([P, 1], F32)
            nc.vector.memset(eps_t[:], 1e-5)

            # ---- resident transposed weights ----
            # WlT: [h-part, h-blk, l] bf16 (8 MB), written by tile-0 prep
            WlT = wbig.tile([P, HBLK, L], BF16, tag="wbig", name="WlT")

            def prep_wl_chunk(m):
                # one 128-row chunk of W_large -> WlT bf16 columns m*P
                wtmp = wprep.tile([P, H], F32, tag="wtmp", name="wtmp")
                nc.sync.dma_start(wtmp[:], W_large[m * P:(m + 1) * P, :])
                wbf = wprep.tile([P, H], BF16, tag="wbf", name="wbf")
                nc.vector.tensor_tensor(
                    wbf[:].rearrange("p (b c) -> p b c", c=P),
                    wtmp[:].rearrange("p (b c) -> p b c", c=P),
                    sc_l[:, m * HBLK:(m + 1) * HBLK, None].to_broadcast([P, HBLK, P]),
                    MUL,
                )
                nc.sync.dma_start_transpose(WlT[:, :, m * P:(m + 1) * P], wbf[:])

            def prep_wc1_chunk(c):
                # one 128-row (l2) chunk of W_c1 -> DRAM bf16/fp8 stationary cache
                for q in range(4):
                    b0 = q * 8  # first l1-block of this quarter
                    wtmp = wprep.tile([P, H], F32, tag="wtmp", name="c1tmp")
                    nc.sync.dma_start(wtmp[:], W_c1[c * P:(c + 1) * P, b0 * P:(b0 + 8) * P])
                    wbf = wprep.tile([P, H], BF16, tag="wbf", name="c1bf")
                    nc.vector.tensor_scalar_mul(wbf[:], wtmp[:], SWC1)
                    wtb = wprep.tile([P, 8, P], BF16, tag="wtb", name="c1tb")
                    nc.sync.dma_start_transpose(wtb[:], wbf[:])
                    nl = max(0, min(8, NLOW - b0))  # blocks of this quarter in bf16
                    if nl > 0:
                        nc.sync.dma_start(WC1B[c][:, b0:b0 + nl, :], wtb[:, :nl, :])
                    if nl < 8:
                        wu = wprep.tile([P, 8 - nl, P], FP8, tag="wuq", name="c1u")
                        nc.scalar.activation(wu[:], wtb[:, nl:, :], Copy)
                        nc.sync.dma_start(WC1U[c][:, b0 + nl - NLOW:b0 + 8 - NLOW, :], wu[:])

            def prep_wc2_bf():
                # W_c2 f32 -> bf16*SWC2 DRAM scratch
                for ch in range(H // P):
                    for hf in range(4):
                        wtmp = wprep.tile([P, H], F32, tag="wtmp", name="c2tmp")
                        nc.sync.dma_start(
                            wtmp[:], W_c2[ch * P:(ch + 1) * P, hf * H:(hf + 1) * H])
                        wbf = wprep.tile([P, H], BF16, tag="wbf", name="c2bf")
                        nc.vector.tensor_scalar_mul(wbf[:], wtmp[:], SWC2)
                        nc.sync.dma_start(
                            Wc2_bf[ch * P:(ch + 1) * P, hf * H:(hf + 1) * H], wbf[:])

            def prep_ws_bf():
                # Ws = (W_s1*s_s1 + W_s2*s_s2)*SWS -> bf16 DRAM scratch
                for ch in range(H // P):
                    w1 = wprep.tile([P, H], F32, tag="wtmp", name="ws1")
                    nc.sync.dma_start(w1[:], W_s1[ch * P:(ch + 1) * P, :])
                    w2 = wprep.tile([P, H], F32, tag="wtmp2", name="ws2", bufs=1)
                    nc.sync.dma_start(w2[:], W_s2[ch * P:(ch + 1) * P, :])
                    t1 = wprep.tile([P, H], BF16, tag="wbf", name="wst1")
                    nc.vector.tensor_tensor(
                        t1[:].rearrange("p (b c) -> p b c", c=P),
                        w1[:].rearrange("p (b c) -> p b c", c=P),
                        sc_s1[:, ch * HBLK:(ch + 1) * HBLK, None].to_broadcast([P, HBLK, P]),
                        MUL,
                    )
                    t2 = wprep.tile([P, H], BF16, tag="wbf2", name="wst2", bufs=1)
                    nc.vector.tensor_tensor(
                        t2[:].rearrange("p (b c) -> p b c", c=P),
                        w2[:].rearrange("p (b c) -> p b c", c=P),
                        sc_s2[:, ch * HBLK:(ch + 1) * HBLK, None].to_broadcast([P, HBLK, P]),
                        MUL,
                    )
                    ts = wprep.tile([P, H], BF16, tag="wbf3", name="wsts", bufs=1)
                    nc.vector.tensor_tensor(ts[:], t1[:], t2[:], ADD)
                    nc.sync.dma_start(Ws_bf[ch * P:(ch + 1) * P, :], ts[:])

            # C-phase residents (Wc2T reuses WlT's slot after phase A ends)
            Wc2T = [None]
            WsT = wsp.tile([P, 2, HBLK, TT], BF16, tag="wsp", name="WsT")

            def prep_wc2T():
                # [l2p, hh, l2blk, h512] bf16; quarters per half
                Wc2T[0] = wbig.tile([P, 2, LBLK, TT], BF16, tag="wbig", name="Wc2T")
                for hh in range(2):
                    for lf in range(4):
                        nc.sync.dma_start_transpose(
                            Wc2T[0][:, hh, lf * HBLK:(lf + 1) * HBLK, :],
                            Wc2_bf[hh * TT:(hh + 1) * TT, lf * H:(lf + 1) * H])

            def prep_wsT():
                for hh in range(2):
                    nc.sync.dma_start_transpose(
                        WsT[:, hh, :, :], Ws_bf[hh * TT:(hh + 1) * TT, :])

            # ================= AB tiles =================
            for t in range(ntt):
                first = t == 0
                par = t % 2

                xT = xp.tile([P, HBLK, TT], BF16, tag=f"xt{par}", name="xT")
                nc.sync.dma_start_transpose(xT[:], x_bf[t * TT:(t + 1) * TT, :])

                # ---- phase A ----
                y1bf = yp.tile([P, LBLK, TT], BF16, tag="ybig", name="y1bf")
                y1f8 = yp.tile([P, NUP // 2, TT, 2], FP8, tag="y1f8", name="y1f8")
                with nc.named_scope(f"A{t}"):
                    for m in range(LBLK):
                        if first:
                            prep_wl_chunk(m)
                        msl = slice(m * P, (m + 1) * P)
                        ps = psum.tile([P, TT], F32, tag="ps", name="psA")
                        for k in range(HBLK):
                            nc.tensor.matmul(
                                ps[:], WlT[:, k, msl], xT[:, k, :],
                                start=(k == 0), stop=(k == HBLK - 1))
                        if m < NLOW:
                            nc.scalar.activation(y1bf[:, m, :], ps[:], Relu, scale=EA)
                        else:
                            mu = m - NLOW
                            nc.scalar.activation(
                                y1f8[:, mu // 2, :, mu % 2], ps[:], Relu, scale=EA)

                # ---- phase B ----
                with nc.named_scope(f"B{t}"):
                    for c in range(LBLK):
                        if first:
                            prep_wc1_chunk(c)
                        if t == min(1, ntt - 1) and c == 0:
                            prep_wc2_bf()
                        if t == max(ntt - 2, 0) and c == 16:
                            prep_ws_bf()
                        if t == ntt - 1:
                            if c == 8:
                                prep_wc2T()
                            if c == 24:
                                prep_wsT()
                        w1b = wc1s.tile([P, NLOW, P], BF16, tag="w1b", name="w1b")
                        nc.sync.dma_start(w1b[:], WC1B[c])
                        w1u = wc1s.tile([P, NUP, P], FP8, tag="w1u", name="w1u")
                        nc.sync.dma_start(w1u[:], WC1U[c])
                        ps = psum.tile([P, TT], F32, tag="ps", name="psB")
                        for j in range(NLOW):
                            nc.tensor.matmul(
                                ps[:], w1b[:, j, :], y1bf[:, j, :],
                                start=(j == 0), stop=False)
                        for jp in range(NUP // 2):
                            nc.tensor.matmul(
                                ps[:], w1u[:, 2 * jp:2 * jp + 2, :],
                                y1f8[:, jp, :, :].rearrange("p t i -> p i t"),
                                start=False, stop=(jp == NUP // 2 - 1), perf_mode=DR)
                        y2s = stg.tile([P, TT], BF16, tag="y2s", name="y2s")
                        nc.scalar.activation(y2s[:], ps[:], Relu, scale=EB)
                        nc.sync.dma_start(Y2B[t][:, c, :], y2s[:])

            # ================= C tiles: y3 + skips + layernorm =================
            for t in range(ntt):
                par = t % 2
                y2bf = yp.tile([P, LBLK, TT], BF16, tag="ybig", name="y2bf")
                nc.sync.dma_start(y2bf[:], Y2B[t])
                xT = xp.tile([P, HBLK, TT], BF16, tag=f"xt{par}", name="cxT")
                nc.sync.dma_start_transpose(xT[:], x_bf[t * TT:(t + 1) * TT, :])
                with nc.named_scope(f"C{t}"):
                    for tn in range(TT // P):
                        tsl = slice(tn * P, (tn + 1) * P)
                        preln = lnp.tile([P, H], FP16, tag="preln", name="preln")
                        pss = [psum.tile([P, TT], F32, tag="ps", name="psC")
                               for _ in range(2)]
                        for j in range(LBLK):
                            for hh, ps in enumerate(pss):
                                nc.tensor.matmul(
                                    ps[:], y2bf[:, j, tsl], Wc2T[0][:, hh, j, :],
                                    start=(j == 0), stop=False)
                        for k in range(HBLK):
                            for hh, ps in enumerate(pss):
                                nc.tensor.matmul(
                                    ps[:], xT[:, k, tsl], WsT[:, hh, k, :],
                                    start=False, stop=(k == HBLK - 1))
                        for hh, ps in enumerate(pss):
                            nc.scalar.activation(
                                preln[:, hh * TT:(hh + 1) * TT], ps[:], Copy, scale=EC)
                        # layernorm over h
                        st = lnp.tile([P, 2, 6], F32, tag="st", name="st")
                        for g in range(2):
                            nc.vector.bn_stats(st[:, g, :], preln[:, g * 512:(g + 1) * 512])
                        mv = lnp.tile([P, 2], F32, tag="mv", name="mv")
                        nc.vector.bn_aggr(mv[:], st[:])
                        std = lnp.tile([P, 1], F32, tag="std", name="std")
                        nc.scalar.activation(std[:], mv[:, 1:2], Sqrt, bias=eps_t[:])
                        rstd = lnp.tile([P, 1], F32, tag="rstd", name="rstd")
                        nc.vector.reciprocal(rstd[:], std[:])
                        og = lnp.tile([P, H], F32, tag="og", name="og")
                        if identity_ln:
                            nc.vector.tensor_scalar(
                                og[:], preln[:], scalar1=mv[:, 0:1], scalar2=rstd[:],
                                op0=SUB, op1=MUL)
                        else:
                            nc.vector.tensor_scalar(
                                preln[:], preln[:], scalar1=mv[:, 0:1], scalar2=rstd[:],
                                op0=SUB, op1=MUL)
                            nc.vector.tensor_tensor(preln[:], preln[:], gamma_b[:], MUL)
                            nc.vector.tensor_tensor(og[:], preln[:], beta_b[:], ADD)
                        nc.sync.dma_start(out[t * TT + tn * P:t * TT + (tn + 1) * P, :], og[:])

    nc.finalize()
    return nc


def _get_nc(identity_ln=True):
    key = (T // TT, identity_ln)
    if key not in _CACHED:
        _CACHED[key] = _build_nc(T // TT, identity_ln)
    return _CACHED[key]


def kernel(**inputs) -> np.ndarray:
    from concourse.bass_utils import run_bass_kernel_spmd

    x = np.asarray(inputs["x"], dtype=np.float32)          # [8, 4096, 1024]
    gamma_np = np.asarray(inputs["gamma"], dtype=np.float32)
    beta_np = np.asarray(inputs["beta"], dtype=np.float32)
    identity_ln = bool(np.all(gamma_np == 1.0) and np.all(beta_np == 0.0))
    nc = _get_nc(identity_ln)
    shared_names = (
        "W_large", "W_s1", "W_s2", "W_c1", "W_c2",
        "gamma", "beta", "s_large", "s_s1", "s_s2",
    )
    shared = {k: np.ascontiguousarray(np.asarray(inputs[k], dtype=np.float32))
              for k in shared_names}
    in_maps = [
        {"x": np.ascontiguousarray(x[c]), **shared} for c in range(NCORES)
    ]
    res = run_bass_kernel_spmd(nc, in_maps, core_ids=list(range(NCORES)))
    return np.stack([res.results[c]["out"] for c in range(NCORES)], axis=0)
